# revision 5
# baseline (speedup 1.0000x reference)
"""Trainium2 Bass kernel for nn_Cov_EBFLayer.

Math: out[b,o] = exp(-quad[o,b]),
  quad[o,b] = diff^T P_o diff,  diff = c_o - x_b,  P_o = B_o B_o^T  (PSD Gram)
            = x^T P x - 2 v_o^T x + q3_o,   v = P c,  q3 = c^T P c
            = sum_{d,f} P[o,d,f] * (x_d x_f)  - 2 sum_d v[o,d] x_d + q3_o

Kernel strategy (per core, batch-sharded 8 x 1024):
  - Degree-2 feature map: G^T[(d,f), b] = x_d * x_f built on DVE from a
    PE-broadcast operand (indicator matmuls) times a stacked xT operand.
  - P computed on device: 256 Gram matmuls betasT_o^T @ betasT_o -> PSUM,
    ACT copies to SBUF in [d, (f,o)] layout, DRAM round trip re-reads it as
    weight chunks W_c[(d,f), o] (contiguous per partition).
  - Main contraction: 33 accumulating matmuls per (o-half, b-tile) PSUM tile:
    32 quadratic chunks (K=128) + 1 augmented chunk (K=65: linear + const).
  - Epilogue: one ACT Exp (scale=-1) straight out of PSUM, DMA out as [O, Bsh].
Host does layout-only prep (transposes) + the tiny linear-term prep
(w = B^T c, v = B w, q3 = w.w : ~2M MACs = 0.01% of model FLOPs).
"""

import sys
from contextlib import ExitStack

import numpy as np

sys.path.insert(0, "/opt/trn_rl_repo")

import concourse.bass as bass  # noqa: E402
import concourse.tile as tile  # noqa: E402
from concourse import bacc, mybir  # noqa: E402
from concourse import bass_utils  # noqa: E402
from concourse._compat import with_exitstack  # noqa: E402

B, D, O, NCORES = 8192, 64, 256, 8
BSH = B // NCORES  # 1024 per-core batch shard
NQC = D // 2  # 32 quadratic chunks, each (2 d's) x (64 f's) = 128 partitions
BT = 512  # b-tile (one PSUM bank of fp32)
NBT = BSH // BT  # 2
F32 = mybir.dt.float32


@with_exitstack
def _kernel(ctx: ExitStack, tc, outT, xT, betasT, indc, lin):
    nc = tc.nc

    cpool = ctx.enter_context(tc.tile_pool(name="const", bufs=1))
    gpool = ctx.enter_context(tc.tile_pool(name="gtiles", bufs=6))
    opool = ctx.enter_context(tc.tile_pool(name="outs", bufs=4))
    dpool = ctx.enter_context(tc.tile_pool(name="dram", bufs=1, space="DRAM"))
    ppool = ctx.enter_context(tc.tile_pool(name="psum_p", bufs=2, space="PSUM"))
    apool = ctx.enter_context(tc.tile_pool(name="psum_a", bufs=2, space="PSUM"))
    qpool = ctx.enter_context(tc.tile_pool(name="psum_q", bufs=4, space="PSUM"))

    # ---- resident inputs ----
    sb_betasT = cpool.tile([D, O * D], F32)  # [e, (o,d)]
    nc.sync.dma_start(sb_betasT[:], betasT[:])
    xb = cpool.tile([128, BSH], F32)  # [xT; xT] stacked
    nc.sync.dma_start(xb[0:D, :], xT[:])
    nc.sync.dma_start(xb[D : 2 * D, :], xT[:])
    sb_indc = cpool.tile([D, NQC * 128], F32)
    nc.sync.dma_start(sb_indc[:], indc[:])
    g_aug = cpool.tile([D + 1, BSH], F32)  # [xT; ones]
    nc.sync.dma_start(g_aug[0:D, :], xT[:])
    nc.gpsimd.memset(g_aug[D : D + 1, :], 1.0)
    w_aug = cpool.tile([D + 1, O], F32)  # [-2 v^T; q3]
    nc.sync.dma_start(w_aug[:], lin[:])

    # ---- phase P: P_o = B_o^T B_o  (Gram), to SBUF layout [d, (f, o)] ----
    # processed in two o-halves so the DRAM round trip pipelines
    p_sb = cpool.tile([D, D * O], F32)  # [d, (f, o)]
    p_sb_v = p_sb[:].rearrange("d (f o) -> d o f", o=O)  # iter (o, f)
    p_dram = dpool.tile([D, D * O], F32)
    p_dram_v = p_dram[:].rearrange("d (f o) -> d f o", o=O)
    w_big = cpool.tile([128, NQC * O], F32)  # chunk c at cols [c*O, (c+1)*O)

    for half in range(2):
        for blk in range(16):  # 8 o's per PSUM bank
            pp = ppool.tile([D, 8 * D], F32)
            for i in range(8):
                o = half * 128 + blk * 8 + i
                bsl = sb_betasT[:, o * D : (o + 1) * D]
                nc.tensor.matmul(
                    pp[:, i * D : (i + 1) * D], bsl, bsl, start=True, stop=True
                )
            # strided copy PSUM -> SBUF: p_sb[d, f*O + o]
            o0 = half * 128 + blk * 8
            nc.scalar.activation(
                p_sb_v[:, o0 : o0 + 8, :],
                pp[:].rearrange("d (i f) -> d i f", f=D),
                mybir.ActivationFunctionType.Copy,
            )
        # round trip through DRAM for this o-half: cols f*O + (half*128..+128)
        o0 = half * 128
        p_sb_fo = p_sb[:].rearrange("d (f o) -> d f o", o=O)
        nc.sync.dma_start(p_dram_v[:, :, o0 : o0 + 128], p_sb_fo[:, :, o0 : o0 + 128])
        for c in range(NQC):
            # dst (p=(j,f), o) <- P[o, 2c+j, f] = p_dram[2c+j][f*O + o]
            nc.sync.dma_start(
                w_big[:, c * O + o0 : c * O + o0 + 128],
                p_dram_v[2 * c : 2 * c + 2, :, o0 : o0 + 128],
            )

    # ---- main: G chunks + accumulating matmuls ----
    pq = {}
    for oh in range(2):
        for bt in range(NBT):
            pq[(oh, bt)] = qpool.tile(
                [128, BT], F32, name=f"pq_{oh}_{bt}", tag="pq"
            )

    for c in range(NQC + 1):
        for bt in range(NBT):
            if c < NQC:
                pa = apool.tile([128, BT], F32)
                nc.tensor.matmul(
                    pa[:],
                    sb_indc[:, c * 128 : (c + 1) * 128],
                    xb[0:D, bt * BT : (bt + 1) * BT],
                    start=True,
                    stop=True,
                )
                g = gpool.tile([128, BT], F32, tag="g")
                nc.vector.tensor_mul(g[:], pa[:], xb[:, bt * BT : (bt + 1) * BT])
                rhs = g[:]
            else:
                rhs = g_aug[:, bt * BT : (bt + 1) * BT]
            for oh in range(2):
                if c < NQC:
                    lhsT = w_big[:, c * O + oh * 128 : c * O + (oh + 1) * 128]
                else:
                    lhsT = w_aug[:, oh * 128 : (oh + 1) * 128]
                nc.tensor.matmul(
                    pq[(oh, bt)][:],
                    lhsT,
                    rhs,
                    start=(c == 0),
                    stop=(c == NQC),
                )

    # ---- epilogue: out = exp(-quad) ----
    for oh in range(2):
        for bt in range(NBT):
            ob = opool.tile([128, BT], F32)
            nc.scalar.activation(
                ob[:],
                pq[(oh, bt)][:],
                mybir.ActivationFunctionType.Exp,
                scale=-1.0,
            )
            nc.sync.dma_start(
                outT[oh * 128 : (oh + 1) * 128, bt * BT : (bt + 1) * BT], ob[:]
            )


_CACHE = {}


def _build():
    if "nc" in _CACHE:
        return _CACHE["nc"], _CACHE["aps"]
    nc = bacc.Bacc(
        "TRN2", target_bir_lowering=False, debug=False, num_devices=NCORES
    )
    xT = nc.dram_tensor("xT", [D, BSH], F32, kind="ExternalInput").ap()
    betasT = nc.dram_tensor("betasT", [D, O * D], F32, kind="ExternalInput").ap()
    indc = nc.dram_tensor("indc", [D, NQC * 128], F32, kind="ExternalInput").ap()
    lin = nc.dram_tensor("lin", [D + 1, O], F32, kind="ExternalInput").ap()
    outT = nc.dram_tensor("outT", [O, BSH], F32, kind="ExternalOutput").ap()
    with tile.TileContext(nc) as tc:
        _kernel(tc, outT, xT, betasT, indc, lin)
    nc.compile()
    _CACHE["nc"] = nc
    _CACHE["aps"] = (xT, betasT, indc, lin, outT)
    return nc, _CACHE["aps"]


def _host_prep(x, centers, betas):
    x = np.asarray(x, np.float32)
    betas = np.asarray(betas, np.float32)
    c = np.asarray(centers, np.float32).reshape(O, D)
    # layout-only transposes
    betasT = np.ascontiguousarray(betas.transpose(2, 0, 1).reshape(D, O * D))
    # indicator constant for PE row-broadcast: indc[d, c*128+p] = [d == 2c + p//64]
    dgrid = 2 * (np.arange(NQC)[:, None] * 1) + (np.arange(128)[None, :] // D)
    indc = (np.arange(D)[:, None, None] == dgrid[None, :, :]).astype(np.float32)
    indc = np.ascontiguousarray(indc.reshape(D, NQC * 128))
    # tiny linear-term prep: w = B^T c, v = B w, q3 = w.w  (~2M MACs)
    w = np.einsum("ofe,of->oe", betas, c)
    v = np.einsum("ode,oe->od", betas, w)
    q3 = np.einsum("oe,oe->o", w, w)
    lin = np.concatenate([-2.0 * v.T, q3[None, :]], axis=0).astype(np.float32)
    lin = np.ascontiguousarray(lin)
    xT_shards = [
        np.ascontiguousarray(x[i * BSH : (i + 1) * BSH].T) for i in range(NCORES)
    ]
    return xT_shards, betasT, indc, lin


def _run(x, centers, betas, trace=False):
    nc, (xT, betasT_ap, indc_ap, lin_ap, outT) = _build()
    xT_shards, betasT, indc, lin = _host_prep(x, centers, betas)
    in_maps = [
        {
            xT.name: xT_shards[i],
            betasT_ap.name: betasT,
            indc_ap.name: indc,
            lin_ap.name: lin,
        }
        for i in range(NCORES)
    ]
    res = bass_utils.run_bass_kernel_spmd(
        nc, in_maps, core_ids=list(range(NCORES)), trace=trace
    )
    out = np.concatenate(
        [np.asarray(res.results[i][outT.name]).T for i in range(NCORES)], axis=0
    )
    return out.astype(np.float32), res


def kernel(x, centers, betas):
    out, _ = _run(x, centers, betas, trace=False)
    return out


# revision 6
# speedup vs baseline: 1.6290x; 1.6290x over previous
"""Trainium2 Bass kernel for nn_Cov_EBFLayer.

Math: out[b,o] = exp(-quad[o,b]),
  quad[o,b] = diff^T P_o diff,  diff = c_o - x_b,  P_o = B_o B_o^T  (PSD Gram)
            = x^T P x - 2 v_o^T x + q3_o,   v = P c,  q3 = c^T P c
            = sum_{d,f} P[o,d,f] * (x_d x_f)  - 2 sum_d v[o,d] x_d + q3_o

Kernel strategy (per core, batch-sharded 8 x 1024):
  - Degree-2 feature map: G^T[(d,f), b] = x_d * x_f built on DVE from a
    PE-broadcast operand (indicator matmuls) times a stacked xT operand.
  - P computed on device: 256 Gram matmuls betasT_o^T @ betasT_o -> PSUM,
    ACT copies to SBUF in [d, (f,o)] layout, DRAM round trip re-reads it as
    weight chunks W_c[(d,f), o] (contiguous per partition).
  - Main contraction: 33 accumulating matmuls per (o-half, b-tile) PSUM tile:
    32 quadratic chunks (K=128) + 1 augmented chunk (K=65: linear + const).
  - Epilogue: one ACT Exp (scale=-1) straight out of PSUM, DMA out as [O, Bsh].
Host does layout-only prep (transposes) + the tiny linear-term prep
(w = B^T c, v = B w, q3 = w.w : ~2M MACs = 0.01% of model FLOPs).
"""

import sys
from contextlib import ExitStack

import numpy as np

sys.path.insert(0, "/opt/trn_rl_repo")

import concourse.bass as bass  # noqa: E402
import concourse.tile as tile  # noqa: E402
from concourse import bacc, mybir  # noqa: E402
from concourse import bass_utils  # noqa: E402
from concourse._compat import with_exitstack  # noqa: E402

B, D, O, NCORES = 8192, 64, 256, 8
BSH = B // NCORES  # 1024 per-core batch shard
NQC = D // 2  # 32 quadratic chunks, each (2 d's) x (64 f's) = 128 partitions
BT = 512  # b-tile (one PSUM bank of fp32)
NBT = BSH // BT  # 2
F32 = mybir.dt.float32
F16 = mybir.dt.float16


@with_exitstack
def _kernel(ctx: ExitStack, tc, outT, xT, betasT, indc, lin):
    nc = tc.nc

    cpool = ctx.enter_context(tc.tile_pool(name="const", bufs=1))
    gpool = ctx.enter_context(tc.tile_pool(name="gtiles", bufs=6))
    opool = ctx.enter_context(tc.tile_pool(name="outs", bufs=4))
    dpool = ctx.enter_context(tc.tile_pool(name="dram", bufs=1, space="DRAM"))
    ppool = ctx.enter_context(tc.tile_pool(name="psum_p", bufs=2, space="PSUM"))
    apool = ctx.enter_context(tc.tile_pool(name="psum_a", bufs=2, space="PSUM"))
    qpool = ctx.enter_context(tc.tile_pool(name="psum_q", bufs=4, space="PSUM"))

    # ---- resident inputs ----
    sb_betasT = cpool.tile([D, O * D], F16)  # [e, (o,d)]
    nc.sync.dma_start(sb_betasT[:], betasT[:])
    xb = cpool.tile([128, BSH], F16)  # [xT; xT] stacked
    nc.sync.dma_start(xb[0:D, :], xT[:])
    nc.sync.dma_start(xb[D : 2 * D, :], xT[:])
    sb_indc = cpool.tile([D, NQC * 128], F16)
    nc.sync.dma_start(sb_indc[:], indc[:])
    g_aug = cpool.tile([D + 1, BSH], F16)  # [xT; ones]
    nc.sync.dma_start(g_aug[0:D, :], xT[:])
    nc.gpsimd.memset(g_aug[D : D + 1, :], 1.0)
    w_aug = cpool.tile([D + 1, O], F16)  # [-2 v^T; q3]
    nc.sync.dma_start(w_aug[:], lin[:])

    # ---- phase P: P_o = B_o^T B_o  (Gram), to SBUF layout [d, (f, o)] ----
    # processed in two o-halves so the DRAM round trip pipelines
    p_sb = cpool.tile([D, D * O], F16)  # [d, (f, o)]
    p_sb_v = p_sb[:].rearrange("d (f o) -> d o f", o=O)  # iter (o, f)
    p_dram = dpool.tile([D, D * O], F16)
    p_dram_v = p_dram[:].rearrange("d (f o) -> d f o", o=O)
    w_big = cpool.tile([128, NQC * O], F16)  # chunk c at cols [c*O, (c+1)*O)

    for half in range(2):
        for blk in range(16):  # 8 o's per PSUM bank
            pp = ppool.tile([D, 8 * D], F32)
            for i in range(8):
                o = half * 128 + blk * 8 + i
                bsl = sb_betasT[:, o * D : (o + 1) * D]
                nc.tensor.matmul(
                    pp[:, i * D : (i + 1) * D], bsl, bsl, start=True, stop=True
                )
            # strided copy PSUM -> SBUF: p_sb[d, f*O + o]
            o0 = half * 128 + blk * 8
            nc.scalar.activation(
                p_sb_v[:, o0 : o0 + 8, :],
                pp[:].rearrange("d (i f) -> d i f", f=D),
                mybir.ActivationFunctionType.Copy,
            )
        # round trip through DRAM for this o-half: cols f*O + (half*128..+128)
        o0 = half * 128
        p_sb_fo = p_sb[:].rearrange("d (f o) -> d f o", o=O)
        nc.sync.dma_start(p_dram_v[:, :, o0 : o0 + 128], p_sb_fo[:, :, o0 : o0 + 128])
        for c in range(NQC):
            # dst (p=(j,f), o) <- P[o, 2c+j, f] = p_dram[2c+j][f*O + o]
            nc.sync.dma_start(
                w_big[:, c * O + o0 : c * O + o0 + 128],
                p_dram_v[2 * c : 2 * c + 2, :, o0 : o0 + 128],
            )

    # ---- main: G chunks + accumulating matmuls ----
    pq = {}
    for oh in range(2):
        for bt in range(NBT):
            pq[(oh, bt)] = qpool.tile(
                [128, BT], F32, name=f"pq_{oh}_{bt}", tag="pq"
            )

    for c in range(NQC + 1):
        for bt in range(NBT):
            if c < NQC:
                pa = apool.tile([128, BT], F32)
                nc.tensor.matmul(
                    pa[:],
                    sb_indc[:, c * 128 : (c + 1) * 128],
                    xb[0:D, bt * BT : (bt + 1) * BT],
                    start=True,
                    stop=True,
                )
                g = gpool.tile([128, BT], F16, tag="g")
                nc.vector.tensor_mul(g[:], pa[:], xb[:, bt * BT : (bt + 1) * BT])
                rhs = g[:]
            else:
                rhs = g_aug[:, bt * BT : (bt + 1) * BT]
            for oh in range(2):
                if c < NQC:
                    lhsT = w_big[:, c * O + oh * 128 : c * O + (oh + 1) * 128]
                else:
                    lhsT = w_aug[:, oh * 128 : (oh + 1) * 128]
                nc.tensor.matmul(
                    pq[(oh, bt)][:],
                    lhsT,
                    rhs,
                    start=(c == 0),
                    stop=(c == NQC),
                )

    # ---- epilogue: out = exp(-quad) ----
    for oh in range(2):
        for bt in range(NBT):
            ob = opool.tile([128, BT], F32)
            nc.scalar.activation(
                ob[:],
                pq[(oh, bt)][:],
                mybir.ActivationFunctionType.Exp,
                scale=-1.0,
            )
            nc.sync.dma_start(
                outT[oh * 128 : (oh + 1) * 128, bt * BT : (bt + 1) * BT], ob[:]
            )


_CACHE = {}


def _build():
    if "nc" in _CACHE:
        return _CACHE["nc"], _CACHE["aps"]
    nc = bacc.Bacc(
        "TRN2", target_bir_lowering=False, debug=False, num_devices=NCORES
    )
    xT = nc.dram_tensor("xT", [D, BSH], F16, kind="ExternalInput").ap()
    betasT = nc.dram_tensor("betasT", [D, O * D], F16, kind="ExternalInput").ap()
    indc = nc.dram_tensor("indc", [D, NQC * 128], F16, kind="ExternalInput").ap()
    lin = nc.dram_tensor("lin", [D + 1, O], F16, kind="ExternalInput").ap()
    outT = nc.dram_tensor("outT", [O, BSH], F32, kind="ExternalOutput").ap()
    with tile.TileContext(nc) as tc:
        _kernel(tc, outT, xT, betasT, indc, lin)
    nc.compile()
    _CACHE["nc"] = nc
    _CACHE["aps"] = (xT, betasT, indc, lin, outT)
    return nc, _CACHE["aps"]


def _host_prep(x, centers, betas):
    x = np.asarray(x, np.float32)
    betas = np.asarray(betas, np.float32)
    c = np.asarray(centers, np.float32).reshape(O, D)
    # layout-only transposes
    betasT = np.ascontiguousarray(betas.transpose(2, 0, 1).reshape(D, O * D)).astype(np.float16)
    # indicator constant for PE row-broadcast: indc[d, c*128+p] = [d == 2c + p//64]
    dgrid = 2 * (np.arange(NQC)[:, None] * 1) + (np.arange(128)[None, :] // D)
    indc = (np.arange(D)[:, None, None] == dgrid[None, :, :]).astype(np.float32)
    indc = np.ascontiguousarray(indc.reshape(D, NQC * 128)).astype(np.float16)
    # tiny linear-term prep: w = B^T c, v = B w, q3 = w.w  (~2M MACs)
    w = np.einsum("ofe,of->oe", betas, c)
    v = np.einsum("ode,oe->od", betas, w)
    q3 = np.einsum("oe,oe->o", w, w)
    lin = np.concatenate([-2.0 * v.T, q3[None, :]], axis=0).astype(np.float16)
    lin = np.ascontiguousarray(lin)
    xT_shards = [
        np.ascontiguousarray(x[i * BSH : (i + 1) * BSH].T).astype(np.float16) for i in range(NCORES)
    ]
    return xT_shards, betasT, indc, lin


def _run(x, centers, betas, trace=False):
    nc, (xT, betasT_ap, indc_ap, lin_ap, outT) = _build()
    xT_shards, betasT, indc, lin = _host_prep(x, centers, betas)
    in_maps = [
        {
            xT.name: xT_shards[i],
            betasT_ap.name: betasT,
            indc_ap.name: indc,
            lin_ap.name: lin,
        }
        for i in range(NCORES)
    ]
    res = bass_utils.run_bass_kernel_spmd(
        nc, in_maps, core_ids=list(range(NCORES)), trace=trace
    )
    out = np.concatenate(
        [np.asarray(res.results[i][outT.name]).T for i in range(NCORES)], axis=0
    )
    return out.astype(np.float32), res


def kernel(x, centers, betas):
    out, _ = _run(x, centers, betas, trace=False)
    return out


# revision 7
# speedup vs baseline: 2.2460x; 1.3788x over previous
"""Trainium2 Bass kernel for nn_Cov_EBFLayer.

Math: out[b,o] = exp(-quad[o,b]),
  quad[o,b] = diff^T P_o diff,  diff = c_o - x_b,  P_o = B_o B_o^T  (PSD Gram)
            = x^T P x - 2 v_o^T x + q3_o,   v = P c,  q3 = c^T P c
            = sum_{d,f} P[o,d,f] * (x_d x_f)  - 2 sum_d v[o,d] x_d + q3_o

Kernel strategy (per core, batch-sharded 8 x 1024):
  - Degree-2 feature map: G^T[(d,f), b] = x_d * x_f built on DVE from a
    PE-broadcast operand (indicator matmuls) times a stacked xT operand.
  - P computed on device: 256 Gram matmuls betasT_o^T @ betasT_o -> PSUM,
    ACT copies to SBUF in [d, (f,o)] layout, DRAM round trip re-reads it as
    weight chunks W_c[(d,f), o] (contiguous per partition).
  - Main contraction: 33 accumulating matmuls per (o-half, b-tile) PSUM tile:
    32 quadratic chunks (K=128) + 1 augmented chunk (K=65: linear + const).
  - Epilogue: one ACT Exp (scale=-1) straight out of PSUM, DMA out as [O, Bsh].
Host does layout-only prep (transposes) + the tiny linear-term prep
(w = B^T c, v = B w, q3 = w.w : ~2M MACs = 0.01% of model FLOPs).
"""

import sys
from contextlib import ExitStack

import numpy as np

sys.path.insert(0, "/opt/trn_rl_repo")

import concourse.bass as bass  # noqa: E402
import concourse.tile as tile  # noqa: E402
from concourse import bacc, mybir  # noqa: E402
from concourse import bass_utils  # noqa: E402
from concourse._compat import with_exitstack  # noqa: E402

B, D, O, NCORES = 8192, 64, 256, 8
BSH = B // NCORES  # 1024 per-core batch shard
NQC = D // 2  # 32 quadratic chunks, each (2 d's) x (64 f's) = 128 partitions
BT = 512  # b-tile (one PSUM bank of fp32)
NBT = BSH // BT  # 2
F32 = mybir.dt.float32
F16 = mybir.dt.float16


@with_exitstack
def _kernel(ctx: ExitStack, tc, outT, xT, betasT, indc, lin):
    nc = tc.nc

    cpool = ctx.enter_context(tc.tile_pool(name="const", bufs=1))
    gpool = ctx.enter_context(tc.tile_pool(name="gtiles", bufs=6))
    opool = ctx.enter_context(tc.tile_pool(name="outs", bufs=4))
    dpool = ctx.enter_context(tc.tile_pool(name="dram", bufs=1, space="DRAM"))
    ppool = ctx.enter_context(tc.tile_pool(name="psum_p", bufs=2, space="PSUM"))
    apool = ctx.enter_context(tc.tile_pool(name="psum_a", bufs=2, space="PSUM"))
    qpool = ctx.enter_context(tc.tile_pool(name="psum_q", bufs=4, space="PSUM"))

    # ---- resident inputs ----
    sb_betasT = cpool.tile([D, O * D], F16)  # [e, (o,d)]
    nc.sync.dma_start(sb_betasT[:], betasT[:])
    xb = cpool.tile([128, BSH], F16)  # [xT; xT] stacked
    nc.sync.dma_start(xb[0:D, :], xT[:])
    nc.sync.dma_start(xb[D : 2 * D, :], xT[:])
    sb_indc = cpool.tile([D, NQC * 128], F16)
    nc.sync.dma_start(sb_indc[:], indc[:])
    g_aug = cpool.tile([D + 1, BSH], F16)  # [xT; ones]
    nc.sync.dma_start(g_aug[0:D, :], xT[:])
    nc.gpsimd.memset(g_aug[D : D + 1, :], 1.0)
    w_aug = cpool.tile([D + 1, O], F16)  # [-2 v^T; q3]
    nc.sync.dma_start(w_aug[:], lin[:])

    # ---- PE warm-up: ~3.5us of back-to-back matmuls so HAM reaches K=8/8
    # while input DMAs are still in flight. Results are overwritten by the
    # real accumulation (start=True resets PSUM). ----
    pq = {}
    for oh in range(2):
        for bt in range(NBT):
            pq[(oh, bt)] = qpool.tile(
                [128, BT], F32, name=f"pq_{oh}_{bt}", tag="pq"
            )
    for i in range(16):
        nc.tensor.matmul(
            pq[(i % 2, (i // 2) % 2)][:],
            xb[0:D, 0:128],
            xb[0:D, 0:BT],
            start=True,
            stop=True,
        )

    # ---- phase P: P_o = B_o^T B_o  (Gram), to SBUF layout [d, (f, o)] ----
    # processed in two o-halves so the DRAM round trip pipelines
    p_sb = cpool.tile([D, D * O], F16)  # [d, (f, o)]
    p_sb_v = p_sb[:].rearrange("d (f o) -> d o f", o=O)  # iter (o, f)
    p_dram = dpool.tile([D, D * O], F16)
    p_dram_v = p_dram[:].rearrange("d (f o) -> d f o", o=O)
    w_big = cpool.tile([128, NQC * O], F16)  # chunk c at cols [c*O, (c+1)*O)

    # device o-index (oo) permutation: even real o -> oo=o/2, odd -> oo=128+o/2.
    # Host un-permutes output rows / permutes lin columns to match.
    p_sb_fo = p_sb[:].rearrange("d (f o) -> d f o", o=O)
    for half in range(2):
        for blk in range(16):  # 4 o-pairs (8 real o's) per PSUM bank
            pp = ppool.tile([128, 4 * 128], F32)
            for t in range(4):
                tt = half * 64 + blk * 4 + t  # pair index: covers o = 2tt, 2tt+1
                bsl = sb_betasT[:, tt * 2 * D : (tt * 2 + 2) * D]  # [64, 128]
                nc.tensor.matmul(
                    pp[:, t * 128 : (t + 1) * 128], bsl, bsl, start=True, stop=True
                )
            # diag blocks -> p_sb[d, f*O + oo]; even o's at oo=t, odd at oo=128+t
            t0 = half * 64 + blk * 4
            pv_lo = pp[0:D, :].rearrange("d (t b) -> d b t", b=128)
            pv_hi = pp[D:128, :].rearrange("d (t b) -> d b t", b=128)
            eng = nc.scalar if blk % 2 == 0 else nc.vector
            if blk % 2 == 0:
                eng.activation(
                    p_sb_fo[:, :, t0 : t0 + 4],
                    pv_lo[:, 0:D, :],
                    mybir.ActivationFunctionType.Copy,
                )
                eng.activation(
                    p_sb_fo[:, :, 128 + t0 : 128 + t0 + 4],
                    pv_hi[:, D:128, :],
                    mybir.ActivationFunctionType.Copy,
                )
            else:
                eng.tensor_copy(p_sb_fo[:, :, t0 : t0 + 4], pv_lo[:, 0:D, :])
                eng.tensor_copy(
                    p_sb_fo[:, :, 128 + t0 : 128 + t0 + 4], pv_hi[:, D:128, :]
                )
        # round trip through DRAM for this real-o half: oo runs
        oo_runs = (half * 64, 128 + half * 64)
        for oo0 in oo_runs:
            nc.sync.dma_start(
                p_dram_v[:, :, oo0 : oo0 + 64], p_sb_fo[:, :, oo0 : oo0 + 64]
            )
        # coalesced W reads: one DMA per (j, oo-run): all 32 chunks at once
        p_dram_j = p_dram[:].rearrange("(c j) (f o) -> j f c o", j=2, o=O)
        w_big_v = w_big[:].rearrange("p (c o) -> p c o", o=O)
        for j in range(2):
            for oo0 in oo_runs:
                nc.sync.dma_start(
                    w_big_v[j * D : (j + 1) * D, :, oo0 : oo0 + 64],
                    p_dram_j[j, :, :, oo0 : oo0 + 64],
                )

    # ---- main: G chunks + accumulating matmuls ----
    for c in range(NQC + 1):
        for bt in range(NBT):
            if c < NQC:
                pa = apool.tile([128, BT], F32)
                nc.tensor.matmul(
                    pa[:],
                    sb_indc[:, c * 128 : (c + 1) * 128],
                    xb[0:D, bt * BT : (bt + 1) * BT],
                    start=True,
                    stop=True,
                )
                g = gpool.tile([128, BT], F16, tag="g")
                nc.vector.tensor_mul(g[:], pa[:], xb[:, bt * BT : (bt + 1) * BT])
                rhs = g[:]
            else:
                rhs = g_aug[:, bt * BT : (bt + 1) * BT]
            for oh in range(2):
                if c < NQC:
                    lhsT = w_big[:, c * O + oh * 128 : c * O + (oh + 1) * 128]
                else:
                    lhsT = w_aug[:, oh * 128 : (oh + 1) * 128]
                nc.tensor.matmul(
                    pq[(oh, bt)][:],
                    lhsT,
                    rhs,
                    start=(c == 0),
                    stop=(c == NQC),
                )

    # ---- epilogue: out = exp(-quad) ----
    for oh in range(2):
        for bt in range(NBT):
            ob = opool.tile([128, BT], F32)
            nc.scalar.activation(
                ob[:],
                pq[(oh, bt)][:],
                mybir.ActivationFunctionType.Exp,
                scale=-1.0,
            )
            nc.sync.dma_start(
                outT[oh * 128 : (oh + 1) * 128, bt * BT : (bt + 1) * BT], ob[:]
            )


_CACHE = {}


def _build():
    if "nc" in _CACHE:
        return _CACHE["nc"], _CACHE["aps"]
    nc = bacc.Bacc(
        "TRN2", target_bir_lowering=False, debug=False, num_devices=NCORES
    )
    xT = nc.dram_tensor("xT", [D, BSH], F16, kind="ExternalInput").ap()
    betasT = nc.dram_tensor("betasT", [D, O * D], F16, kind="ExternalInput").ap()
    indc = nc.dram_tensor("indc", [D, NQC * 128], F16, kind="ExternalInput").ap()
    lin = nc.dram_tensor("lin", [D + 1, O], F16, kind="ExternalInput").ap()
    outT = nc.dram_tensor("outT", [O, BSH], F32, kind="ExternalOutput").ap()
    with tile.TileContext(nc) as tc:
        _kernel(tc, outT, xT, betasT, indc, lin)
    nc.compile()
    _CACHE["nc"] = nc
    _CACHE["aps"] = (xT, betasT, indc, lin, outT)
    return nc, _CACHE["aps"]


def _host_prep(x, centers, betas):
    x = np.asarray(x, np.float32)
    betas = np.asarray(betas, np.float32)
    c = np.asarray(centers, np.float32).reshape(O, D)
    # layout-only transposes
    betasT = np.ascontiguousarray(betas.transpose(2, 0, 1).reshape(D, O * D)).astype(np.float16)
    # indicator constant for PE row-broadcast: indc[d, c*128+p] = [d == 2c + p//64]
    dgrid = 2 * (np.arange(NQC)[:, None] * 1) + (np.arange(128)[None, :] // D)
    indc = (np.arange(D)[:, None, None] == dgrid[None, :, :]).astype(np.float32)
    indc = np.ascontiguousarray(indc.reshape(D, NQC * 128)).astype(np.float16)
    # tiny linear-term prep: w = B^T c, v = B w, q3 = w.w  (~2M MACs)
    w = np.einsum("ofe,of->oe", betas, c)
    v = np.einsum("ode,oe->od", betas, w)
    q3 = np.einsum("oe,oe->o", w, w)
    lin = np.concatenate([-2.0 * v.T, q3[None, :]], axis=0).astype(np.float16)
    # device o-permutation: even o -> o//2, odd o -> 128 + o//2
    operm = np.array([o // 2 + (o % 2) * 128 for o in range(O)])
    lin_d = np.empty_like(lin)
    lin_d[:, operm] = lin
    lin = np.ascontiguousarray(lin_d)
    xT_shards = [
        np.ascontiguousarray(x[i * BSH : (i + 1) * BSH].T).astype(np.float16) for i in range(NCORES)
    ]
    return xT_shards, betasT, indc, lin


def _run(x, centers, betas, trace=False):
    nc, (xT, betasT_ap, indc_ap, lin_ap, outT) = _build()
    xT_shards, betasT, indc, lin = _host_prep(x, centers, betas)
    in_maps = [
        {
            xT.name: xT_shards[i],
            betasT_ap.name: betasT,
            indc_ap.name: indc,
            lin_ap.name: lin,
        }
        for i in range(NCORES)
    ]
    res = bass_utils.run_bass_kernel_spmd(
        nc, in_maps, core_ids=list(range(NCORES)), trace=trace
    )
    operm = np.array([o // 2 + (o % 2) * 128 for o in range(O)])
    out = np.concatenate(
        [np.asarray(res.results[i][outT.name])[operm, :].T for i in range(NCORES)],
        axis=0,
    )
    return out.astype(np.float32), res


def kernel(x, centers, betas):
    out, _ = _run(x, centers, betas, trace=False)
    return out
